# revision 1
# baseline (speedup 1.0000x reference)
"""Multi-head causal attention (B=2, S=2048, D=1024, H=16) on 8 TRN2 NeuronCores.

Sharding: Megatron-style head parallelism. Core c owns heads {2c, 2c+1}:
  - W_q/W_k/W_v column slices [:, 128c:128(c+1)]  (2 heads x 64 dims)
  - attention for those heads over the full sequence (causal)
  - normalized context slices are AllGathered across cores (bf16, 4 chunks
    overlapped with attention of later tiles)
  - each core computes the output-projection column slice
    out[:, 128c:128(c+1)] = ctx_full @ W_o[:, 128c:128(c+1)]
  - host concatenates the 8 column slices (pure gather, no arithmetic)

Compute dtype: bf16 operands, fp32 PSUM accumulation. Scores are computed
transposed (S^T[k,q] = K Q^T) so the P^T tiles feed the A@V matmul directly;
softmax denominators come from an extra all-ones column appended to V.
Pipelining: x is cast+transposed per 512-row chunk with QKV projections and
the first attention tiles interleaved, so the TensorEngine never sits idle
behind the DMA pipeline.
"""

import numpy as np

import concourse.bass as bass
import concourse.mybir as mybir
from concourse import bacc, tile
from concourse.masks import make_identity
from concourse.bass_utils import run_bass_kernel_spmd

N_CORES = 8
B, S, D = 2, 2048, 1024
H, DH = 16, 64
BS = B * S  # 4096
HPC = H // N_CORES  # heads per core = 2
DHC = HPC * DH  # 128 context dims per core
SCALE = 1.0 / 32.0  # 1/sqrt(D)
FP32 = mybir.dt.float32
BF16 = mybir.dt.bfloat16
Exp = mybir.ActivationFunctionType.Exp

NQ = 4  # q macro tiles of 512 per batch element
QM = S // NQ  # 512
NKT = S // 128  # 16 k-tiles of 128 per batch element

_nc_cache = {}


def _build():
    nc = bacc.Bacc(
        "TRN2", target_bir_lowering=False, debug=False, num_devices=N_CORES
    )

    x_d = nc.dram_tensor("x", [BS, D], FP32, kind="ExternalInput").ap()
    wq_d = nc.dram_tensor("wq", [D, DHC], FP32, kind="ExternalInput").ap()
    wk_d = nc.dram_tensor("wk", [D, DHC], FP32, kind="ExternalInput").ap()
    wv_d = nc.dram_tensor("wv", [D, DHC], FP32, kind="ExternalInput").ap()
    wo_d = nc.dram_tensor("wo", [D, DHC], FP32, kind="ExternalInput").ap()
    tri_d = nc.dram_tensor("tri", [128, 128], FP32, kind="ExternalInput").ap()
    out_d = nc.dram_tensor("out", [BS, DHC], FP32, kind="ExternalOutput").ap()

    with tile.TileContext(nc) as tc:
        with (
            tc.tile_pool(name="dram", bufs=1, space="DRAM") as dram,
            tc.tile_pool(name="pers", bufs=1) as pers,
            tc.tile_pool(name="ptp", bufs=6) as ptp,
            tc.tile_pool(name="nw", bufs=3) as nw,
            tc.tile_pool(name="ps_s", bufs=2, space="PSUM") as ps_s,
            tc.tile_pool(name="ps_c", bufs=2, space="PSUM") as ps_c,
            tc.tile_pool(name="ps_m", bufs=2, space="PSUM") as ps_m,
        ):
            # ---- persistent SBUF ----
            qt_sb = [pers.tile([128, S], BF16, name=f"qt{b}") for b in range(B)]
            kt_sb = [pers.tile([128, S], BF16, name=f"kt{b}") for b in range(B)]
            # V tiles: per k-tile layout [h0 64 | ones | h1 64 | ones] (130 cols)
            v_sb = [pers.tile([128, NKT * 130], BF16, name=f"v{b}") for b in range(B)]
            wq_sb = pers.tile([128, 8, DHC], BF16, name="wq_sb")
            wk_sb = pers.tile([128, 8, DHC], BF16, name="wk_sb")
            wv_sb = pers.tile([128, 8, DHC], BF16, name="wv_sb")
            wo_sb = pers.tile([128, 8, DHC], BF16, name="wo_sb")
            tri_sb = pers.tile([128, 128], BF16, name="tri_sb")
            ones_sb = pers.tile([1, 64], BF16, name="ones_sb")
            nc.vector.memset(ones_sb[:], 1.0)
            id_b = pers.tile([128, 128], BF16, name="id_b")
            make_identity(nc, id_b[:])
            id_f = pers.tile([128, 128], FP32, name="id_f")
            make_identity(nc, id_f[:])

            # ---- weights: load fp32 (gpsimd queue), cast to bf16 ----
            wtmp = pers.tile([128, 8, DHC], FP32, name="wtmp")
            for w_d, w_sb in ((wq_d, wq_sb), (wk_d, wk_sb), (wv_d, wv_sb), (wo_d, wo_sb)):
                nc.gpsimd.dma_start(
                    wtmp[:], w_d.rearrange("(c p) n -> p c n", p=128)
                )
                nc.vector.tensor_copy(w_sb[:], wtmp[:])
            tri_f = nw.tile([128, 128], FP32, name="tri_f")
            nc.gpsimd.dma_start(tri_f[:], tri_d[:])
            nc.vector.tensor_copy(tri_sb[:], tri_f[:])

            # ---- attention output chunks (one per m tile, both b) ----
            # chunks 0-2: m tile for both b; chunks 3a/3b: m=3 split by b
            ctx_in_c = [
                dram.tile([DHC, 2 * QM], BF16, name=f"ctx_in{k}") for k in range(3)
            ] + [
                dram.tile([DHC, QM], BF16, name=f"ctx_in3{s}") for s in "ab"
            ]
            ctx_all_c = [
                dram.tile(
                    [N_CORES * DHC, 2 * QM], BF16, name=f"ctx_all{k}",
                    addr_space="Shared",
                )
                for k in range(3)
            ] + [
                dram.tile(
                    [N_CORES * DHC, QM], BF16, name=f"ctx_all3{s}",
                    addr_space="Shared",
                )
                for s in "ab"
            ]

            def attention(b, m):
                """Emit the kt loop + PSUM eviction; return deferred norm tail."""
                qcols = slice(m * QM, (m + 1) * QM)
                ctx_ps = [
                    ps_c.tile([65, QM], FP32, name=f"ctx_ps{h}", tag="c")
                    for h in range(HPC)
                ]
                n_kt = 4 * m + 4

                def score_mm(kt):
                    s_ps = ps_s.tile([128, 2 * QM], FP32, name="s_ps", tag="s")
                    for h in range(HPC):
                        nc.tensor.matmul(
                            s_ps[:, h * QM : (h + 1) * QM],
                            kt_sb[b][h * 64 : (h + 1) * 64, kt * 128 : (kt + 1) * 128],
                            qt_sb[b][h * 64 : (h + 1) * 64, qcols],
                            start=True,
                            stop=True,
                            tile_position=(h * 64, 0),
                        )
                    return s_ps

                s_cur = score_mm(0)
                for kt in range(n_kt):
                    s_nxt = score_mm(kt + 1) if kt + 1 < n_kt else None
                    s_ps = s_cur
                    j = kt - 4 * m  # diagonal block index if >= 0
                    qs = max(0, 128 * j)
                    pt = ptp.tile([128, 2 * QM], BF16, name="pt")
                    if j < 0:
                        nc.scalar.activation(pt[:], s_ps[:], Exp, scale=SCALE)
                    else:
                        for h in range(HPC):
                            nc.scalar.activation(
                                pt[:, h * QM + qs : (h + 1) * QM],
                                s_ps[:, h * QM + qs : (h + 1) * QM],
                                Exp,
                                scale=SCALE,
                            )
                            nc.vector.tensor_mul(
                                pt[:, h * QM + qs : h * QM + qs + 128],
                                pt[:, h * QM + qs : h * QM + qs + 128],
                                tri_sb[:],
                            )
                    for h in range(HPC):
                        nc.tensor.matmul(
                            ctx_ps[h][:, qs:QM],
                            v_sb[b][:, kt * 130 + h * 65 : kt * 130 + (h + 1) * 65],
                            pt[:, h * QM + qs : (h + 1) * QM],
                            start=(kt == 0),
                            stop=(kt == n_kt - 1),
                        )
                    s_cur = s_nxt
                # evict accumulators to SBUF now (frees the PSUM banks) and
                # compute reciprocals; the PE-side normalize is deferred so the
                # next attention's score matmuls fill the reciprocal latency.
                ctxa_l, recip_l = [], []
                for h in range(HPC):
                    ctxa = nw.tile([65, QM], FP32, name="ctxa", tag="ctxa", bufs=4)
                    nc.vector.tensor_copy(ctxa[:], ctx_ps[h][:])
                    recip = nw.tile([1, QM], BF16, name="recip", tag="recip", bufs=4)
                    with nc.allow_low_precision(reason="softmax denom to bf16"):
                        nc.vector.reciprocal(recip[:], ctxa[64:65, :])
                    ctxa_l.append(ctxa)
                    recip_l.append(recip)

                def tail():
                    for h in range(HPC):
                        bc_ps = ps_m.tile([128, QM], FP32, name="bc_ps", tag="m")
                        nc.tensor.matmul(
                            bc_ps[0:64, :], ones_sb[:], recip_l[h][:],
                            start=True, stop=True,
                        )
                        ctxn = nw.tile([64, QM], BF16, name="ctxn")
                        nc.vector.tensor_mul(
                            ctxn[:], ctxa_l[h][0:64, :], bc_ps[0:64, :]
                        )
                        if m < 3:
                            dst_ap = ctx_in_c[m][
                                h * 64 : (h + 1) * 64, b * QM : (b + 1) * QM
                            ]
                        else:
                            dst_ap = ctx_in_c[3 + b][h * 64 : (h + 1) * 64, :]
                        nc.scalar.dma_start(dst_ap, ctxn[:])

                return tail

            def allgather(k):
                nc.gpsimd.collective_compute(
                    "AllGather",
                    mybir.AluOpType.bypass,
                    replica_groups=[list(range(N_CORES))],
                    ins=[ctx_in_c[k][:]],
                    outs=[ctx_all_c[k][:]],
                )

            def qkv_chunk(b, j, xt_sb):
                cols = slice(b * S + j * QM, b * S + (j + 1) * QM)
                for w_sb, t_sb in ((wq_sb, qt_sb[b]), (wk_sb, kt_sb[b])):
                    ps = ps_m.tile([128, QM], FP32, name="ps_qk", tag="m")
                    for dt in range(8):
                        nc.tensor.matmul(
                            ps[:],
                            w_sb[:, dt, :],
                            xt_sb[:, dt, cols],
                            start=(dt == 0),
                            stop=(dt == 7),
                        )
                    nc.vector.tensor_copy(t_sb[:, j * QM : (j + 1) * QM], ps[:])
                vt_ps = ps_m.tile([128, QM], FP32, name="vt_ps", tag="m")
                for dt in range(8):
                    nc.tensor.matmul(
                        vt_ps[:],
                        wv_sb[:, dt, :],
                        xt_sb[:, dt, cols],
                        start=(dt == 0),
                        stop=(dt == 7),
                    )
                vt_sb = nw.tile([128, QM], BF16, name="vt_sb", tag="vt", bufs=2)
                nc.vector.tensor_copy(vt_sb[:], vt_ps[:])
                vtr_ps = ps_m.tile([128, QM], BF16, name="vtr_ps", tag="m")
                for st2 in range(4):
                    nc.tensor.transpose(
                        vtr_ps[:, st2 * 128 : (st2 + 1) * 128],
                        vt_sb[:, st2 * 128 : (st2 + 1) * 128],
                        id_b[:],
                    )
                dst = v_sb[b][:, j * 520 : (j + 1) * 520].rearrange(
                    "p (t g c) -> p t g c", t=4, g=2
                )[:, :, :, 0:64]
                vsrc = vtr_ps[:].rearrange("p (t g c) -> p t g c", t=4, g=2)
                nc.vector.tensor_copy(dst, vsrc)

            def outproj(m, cfp):
                cf = cfp.tile([128, 8, 2 * QM], BF16, name="cf", tag="cf", bufs=1)
                for dt in range(8):
                    if m < 3:
                        nc.sync.dma_start(
                            cf[:, dt, :], ctx_all_c[m][dt * 128 : (dt + 1) * 128, :]
                        )
                    else:
                        nc.sync.dma_start(
                            cf[:, dt, 0:QM],
                            ctx_all_c[3][dt * 128 : (dt + 1) * 128, :],
                        )
                        nc.sync.dma_start(
                            cf[:, dt, QM : 2 * QM],
                            ctx_all_c[4][dt * 128 : (dt + 1) * 128, :],
                        )
                o_sb = nw.tile([128, 8, DHC], FP32, name="o_sb", tag="o", bufs=2)
                for bb in range(B):
                    ot_ps = ps_m.tile([128, QM], FP32, name="ot_ps", tag="m")
                    for dt in range(8):
                        nc.tensor.matmul(
                            ot_ps[:],
                            wo_sb[:, dt, :],
                            cf[:, dt, bb * QM : (bb + 1) * QM],
                            start=(dt == 0),
                            stop=(dt == 7),
                        )
                    ot_sb = nw.tile([128, QM], FP32, name="ot_sb", tag="ot", bufs=2)
                    if bb % 2 == 0:
                        nc.scalar.copy(ot_sb[:], ot_ps[:])
                    else:
                        nc.vector.tensor_copy(ot_sb[:], ot_ps[:])
                    otr_ps = ps_m.tile([128, QM], FP32, name="otr_ps", tag="m")
                    for qi in range(4):
                        nc.tensor.transpose(
                            otr_ps[:, qi * 128 : (qi + 1) * 128],
                            ot_sb[:, qi * 128 : (qi + 1) * 128],
                            id_f[:],
                        )
                    if bb % 2 == 0:
                        nc.vector.tensor_copy(
                            o_sb[:, bb * 4 : (bb + 1) * 4, :],
                            otr_ps[:].rearrange("p (c n) -> p c n", c=4),
                        )
                    else:
                        nc.scalar.copy(
                            o_sb[:, bb * 4 : (bb + 1) * 4, :],
                            otr_ps[:].rearrange("p (c n) -> p c n", c=4),
                        )
                for bb in range(B):
                    nc.gpsimd.dma_start(
                        out_d[bb * S + m * QM : bb * S + (m + 1) * QM, :].rearrange(
                            "(c p) n -> p c n", p=128
                        ),
                        o_sb[:, bb * 4 : (bb + 1) * 4, :],
                    )

            # ---- x: load fp32, cast bf16 (vector), write back (scalar queue),
            # transpose-read (sync queue); interleaved with qkv + attention per m
            xbf_dram = dram.tile([BS, D], BF16, name="xbf_dram")
            with (
                tc.tile_pool(name="xtp", bufs=1) as xtp,
                tc.tile_pool(name="ldx", bufs=2) as ldx,
            ):
                xt_sb = xtp.tile([128, 8, BS], BF16, name="xt_sb")
                memset_done = set()

                def x_chunk(b, j):
                    g = b * 4 + j
                    rows = slice(g * 512, (g + 1) * 512)
                    x_f = ldx.tile([128, 4, D], FP32, name="x_f", tag="xf")
                    nc.sync.dma_start(
                        x_f[:], x_d[rows, :].rearrange("(c p) d -> p c d", p=128)
                    )
                    x_b = ldx.tile([128, 4, D], BF16, name="x_b", tag="xb")
                    nc.vector.tensor_copy(x_b[:], x_f[:])
                    if g == 0:
                        # first chunk: PE-transpose directly (skips the DRAM
                        # round-trip + xbar on the critical startup path)
                        for dt in range(8):
                            ps_t = ps_m.tile([128, QM], BF16, name="ps_t", tag="m")
                            for st in range(4):
                                nc.tensor.transpose(
                                    ps_t[:, st * 128 : (st + 1) * 128],
                                    x_b[:, st, dt * 128 : (dt + 1) * 128],
                                    id_b[:],
                                )
                            eng = nc.scalar if dt % 2 == 0 else nc.vector
                            if eng is nc.scalar:
                                eng.copy(
                                    xt_sb[:, dt, g * 512 : (g + 1) * 512], ps_t[:]
                                )
                            else:
                                eng.tensor_copy(
                                    xt_sb[:, dt, g * 512 : (g + 1) * 512], ps_t[:]
                                )
                        return_early = True
                    else:
                        nc.scalar.dma_start(
                            xbf_dram[rows, :].rearrange("(c p) d -> p c d", p=128),
                            x_b[:],
                        )
                        for dt in range(8):
                            nc.sync.dma_start_transpose(
                                xt_sb[:, dt, g * 512 : (g + 1) * 512],
                                xbf_dram[rows, dt * 128 : (dt + 1) * 128],
                            )
                    if b not in memset_done:
                        memset_done.add(b)
                        nc.gpsimd.memset(v_sb[b][:], 1.0)
                    qkv_chunk(b, j, xt_sb)

                for m in range(NQ):
                    x_chunk(0, m)
                    x_chunk(1, m)

            # ---- attention + chunked collective + output projection ----
            with tc.tile_pool(name="cfp", bufs=2) as cfp:
                t00 = attention(0, 0)
                t10 = attention(1, 0)
                t00()
                t01 = attention(0, 1)
                t10()
                allgather(0)
                t11 = attention(1, 1)
                t01()
                t02 = attention(0, 2)
                t11()
                allgather(1)
                t12 = attention(1, 2)
                t02()
                outproj(0, cfp)
                t03 = attention(0, 3)
                t12()
                allgather(2)
                t13 = attention(1, 3)
                t03()
                allgather(3)
                outproj(1, cfp)
                t13()
                allgather(4)
                outproj(2, cfp)
                outproj(3, cfp)

    nc.compile()
    return nc


def _build_nc():
    if "nc" not in _nc_cache:
        _nc_cache["nc"] = _build()
    return _nc_cache["nc"]


def kernel(x, W_q, W_k, W_v, W_o):
    x = np.ascontiguousarray(np.asarray(x, dtype=np.float32)).reshape(BS, D)
    # keep-mask for the diagonal 128x128 block of S^T[k, q]: keep k <= q
    tri = np.triu(np.ones((128, 128), dtype=np.float32))
    in_maps = []
    for c in range(N_CORES):
        sl = slice(c * DHC, (c + 1) * DHC)
        in_maps.append(
            {
                "x": x,
                "wq": np.ascontiguousarray(np.asarray(W_q, np.float32)[:, sl]),
                "wk": np.ascontiguousarray(np.asarray(W_k, np.float32)[:, sl]),
                "wv": np.ascontiguousarray(np.asarray(W_v, np.float32)[:, sl]),
                "wo": np.ascontiguousarray(np.asarray(W_o, np.float32)[:, sl]),
                "tri": tri,
            }
        )
    nc = _build_nc()
    res = run_bass_kernel_spmd(nc, in_maps, core_ids=list(range(N_CORES)))
    out = np.concatenate([res.results[c]["out"] for c in range(N_CORES)], axis=1)
    return out.reshape(B, S, D)

